# revision 8
# baseline (speedup 1.0000x reference)
"""Masked dot-product attention on 8 TRN2 NeuronCores.

Math (per batch b):
    S = Q @ K^T / sqrt(64)                    [SQ, SK]
    S[:, k >= vl_b] = -1e6; A = softmax(S)    (masked cols -> weight 0)
    O = A @ V                                 [SQ, 64]

Device strategy (per core, SPMD — identical instruction stream):
  * scores are computed transposed: S_T[k, q] = sum_d K[k,d] Q[q,d]
    via matmul(lhsT=K^T tile [64,128], rhs=Q^T chunk [64,512]).
  * no max-subtraction: |S/8| <= ~6 so exp never overflows; the
    reference's masked lanes underflow to exactly 0 in fp32, we instead
    zero V rows (host-side) so masked keys contribute 0 to both
    numerator and denominator — identical result, zero device masking
    cost.
  * denominator via ones-column appended to V (host-side):
    O_aug^T[65, q] = sum_k V_aug[k, :]^T * exp(S_T[k, q]) accumulated in
    PSUM over k-tiles; row 64 is the softmax denominator.
  * host does final divide + transpose (tiny), so the device never
    needs cross-partition broadcasts.
  * matmul operands are fp16 (PE streams 2-byte dtypes at full rate;
    4-byte f32r measured 2.6x slower). PSUM accumulation stays fp32.

Work scheduling: the host knows valid_lens at compile time, so each core
receives a host-packed list of (q-chunk "slot", k-tile "unit") work items
covering only k < vl. All cores run the same program shape (same slot/unit
counts, compile-time constants); per-core differences live entirely in the
packed input data. Cores with fewer real k-tiles get padding units whose
V_aug is all-zero (contributes nothing).
"""

import functools
import math

import numpy as np

B, SQ, SK, D = 16, 2048, 2048, 64
NCORES = 8
KT = 128          # k rows per unit (one matmul stationary tile)
QCH = 1024        # q columns per slot
NSLOTS_TOTAL = B * (SQ // QCH)   # 32 slot-items across all cores
SLOTS_PER_CORE = NSLOTS_TOTAL // NCORES  # 4
VA_W = D + 1      # V columns + ones column
PW = KT + 2 * VA_W  # merged pair row width: K^T pair cols + 2x V_aug cols

_last_results = None  # stashed BassKernelResults for test.py introspection


def _nkt(vl: int) -> int:
    return max(1, min(SK // KT, math.ceil(vl / KT)))


def _make_schedule(vl: np.ndarray, full: bool = False):
    """Assign the 32 (batch, q-half) slot-items to 8 cores, balanced by
    k-tile count. Returns (slot_sizes, assign): slot_sizes[s] is the
    compile-time unit count of slot s (same for every core);
    assign[core][s] = (batch, half, n_real_ktiles). Sorted grouping
    minimizes sum-of-group-maxima for fixed group size."""
    w = [SK // KT if full else _nkt(int(vl[b])) for b in range(B)]
    items = [(b, h) for b in range(B) for h in range(SQ // QCH)]
    items.sort(key=lambda t: -w[t[0]])
    slot_sizes = []
    assign = [[] for _ in range(NCORES)]
    for s in range(SLOTS_PER_CORE):
        group = items[NCORES * s : NCORES * s + NCORES]
        u = max(w[b] for b, _ in group)
        u += u % 2  # round to even: units are processed as row-group pairs
        slot_sizes.append(u)
        for c, (b, h) in enumerate(group):
            assign[c].append((b, h, w[b]))
    return tuple(slot_sizes), assign


@functools.lru_cache(maxsize=4)
def _build_program(slot_sizes: tuple):
    """Build + schedule the SPMD Bass program for the given slot shape."""
    import concourse.bacc as bacc
    import concourse.mybir as mybir
    import concourse.tile as tile

    n_units = sum(slot_sizes)
    f32 = mybir.dt.float32
    f16 = mybir.dt.float16

    nc = bacc.Bacc(
        "TRN2",
        target_bir_lowering=False,
        debug=False,
        enable_asserts=False,
        num_devices=NCORES,
    )
    n_pairs = n_units // 2
    qtd = nc.dram_tensor("qtd", [SLOTS_PER_CORE, KT, QCH], f16, kind="ExternalInput")
    uin = nc.dram_tensor("uin", [n_pairs, KT, PW], f16, kind="ExternalInput")
    o = nc.dram_tensor("o", [SLOTS_PER_CORE, VA_W, QCH], f32, kind="ExternalOutput")

    with tile.TileContext(nc) as tc:
        with (
            tc.tile_pool(name="qpool", bufs=2) as qpool,
            tc.tile_pool(name="upool", bufs=6) as upool,
            tc.tile_pool(name="ptpool", bufs=4) as ptpool,
            tc.tile_pool(name="opool", bufs=2) as opool,
            tc.tile_pool(name="scpool", bufs=1, space="PSUM") as scpool,
            tc.tile_pool(name="accpool", bufs=2, space="PSUM") as accpool,
        ):
            p = 0
            for s, nu in enumerate(slot_sizes):
                # Q^T chunk, duplicated into both partition halves so both
                # row-group matmuls of a pair can stream it concurrently.
                qt = qpool.tile([KT, QCH], f16)
                nc.sync.dma_start(out=qt, in_=qtd[s])
                acc = accpool.tile([VA_W, QCH], f32)
                for jp in range(nu // 2):
                    ump = upool.tile([KT, PW], f16)
                    nc.sync.dma_start(out=ump, in_=uin[p])
                    kt_a = ump[0:D, 0:KT]          # unit A K^T, rows 0-63
                    kt_b = ump[D:KT, 0:KT]         # unit B K^T, rows 64-127
                    va_a = ump[:, KT : KT + VA_W]
                    va_b = ump[:, KT + VA_W : PW]
                    # Two k-tiles' scores concurrently via PE row groups
                    # (contraction is only 64 deep; A uses rows 0-63,
                    # B rows 64-127 — tile_position auto-derived from the
                    # operands' base partitions).
                    sc_a = scpool.tile([KT, QCH], f32, tag="sc_a")
                    sc_b = scpool.tile([KT, QCH], f32, tag="sc_b")
                    for c in range(QCH // 512):
                        nc.tensor.matmul(
                            sc_a[:, c * 512 : (c + 1) * 512],
                            lhsT=kt_a,
                            rhs=qt[0:D, c * 512 : (c + 1) * 512],
                            start=True,
                            stop=True,
                        )
                        nc.tensor.matmul(
                            sc_b[:, c * 512 : (c + 1) * 512],
                            lhsT=kt_b,
                            rhs=qt[D:KT, c * 512 : (c + 1) * 512],
                            start=True,
                            stop=True,
                        )
                    pt_a = ptpool.tile([KT, QCH], f16, tag="pt_a")
                    pt_b = ptpool.tile([KT, QCH], f16, tag="pt_b")
                    nc.scalar.activation(
                        pt_a, sc_a, mybir.ActivationFunctionType.Exp,
                        scale=1.0 / math.sqrt(D),
                    )
                    nc.scalar.activation(
                        pt_b, sc_b, mybir.ActivationFunctionType.Exp,
                        scale=1.0 / math.sqrt(D),
                    )
                    for va_t, pt, j in ((va_a, pt_a, 2 * jp), (va_b, pt_b, 2 * jp + 1)):
                        for c in range(QCH // 512):
                            nc.tensor.matmul(
                                acc[:, c * 512 : (c + 1) * 512],
                                lhsT=va_t,
                                rhs=pt[:, c * 512 : (c + 1) * 512],
                                start=(j == 0),
                                stop=(j == nu - 1),
                            )
                    p += 1
                o_sb = opool.tile([VA_W, QCH], f32)
                nc.vector.tensor_copy(o_sb, acc)
                nc.sync.dma_start(out=o[s], in_=o_sb)
    nc.compile()
    return nc


def _pack_inputs(queries, keys, values, vl, slot_sizes, assign):
    """Build each core's packed device inputs per its schedule."""
    n_pairs = sum(slot_sizes) // 2
    qT = np.ascontiguousarray(queries.transpose(0, 2, 1).astype(np.float16))
    kT = keys.astype(np.float16)  # [B, SK, D] row-major, sliced per k-tile
    in_maps = []
    for c in range(NCORES):
        qtd = np.zeros((SLOTS_PER_CORE, KT, QCH), np.float16)
        uin = np.zeros((n_pairs, KT, PW), np.float16)
        p = 0
        for s, nu in enumerate(slot_sizes):
            b, h, w = assign[c][s]
            qtd[s, :D] = qT[b, :, h * QCH : (h + 1) * QCH]
            qtd[s, D:KT] = qtd[s, :D]  # duplicate for row-group B's stream
            nvalid = int(vl[b])
            va = np.zeros((SK, VA_W), np.float16)
            va[:nvalid, :D] = values[b, :nvalid, :]
            va[:nvalid, D] = 1.0
            for jp in range(nu // 2):
                for half, j in ((0, 2 * jp), (1, 2 * jp + 1)):
                    t = min(j, w - 1)  # padding units replay a real k-tile
                    rows = slice(0, D) if half == 0 else slice(D, KT)
                    uin[p, rows, :KT] = kT[b, t * KT : (t + 1) * KT, :].T
                    if j < w:
                        col0 = KT + half * VA_W
                        uin[p, :, col0 : col0 + VA_W] = va[t * KT : (t + 1) * KT, :]
                    # else: V_aug stays zero -> padding unit contributes 0
                p += 1
        in_maps.append({"qtd": qtd, "uin": uin})
    return in_maps


def kernel(queries, keys, values, valid_lens, _full=False, _trace=False):
    global _last_results
    from concourse.bass_utils import run_bass_kernel_spmd

    queries = np.ascontiguousarray(np.asarray(queries, dtype=np.float32))
    keys = np.ascontiguousarray(np.asarray(keys, dtype=np.float32))
    values = np.ascontiguousarray(np.asarray(values, dtype=np.float32))
    vl = np.asarray(valid_lens).astype(np.int64).reshape(B)

    slot_sizes, assign = _make_schedule(vl, full=_full)
    nc = _build_program(slot_sizes)
    in_maps = _pack_inputs(queries, keys, values, vl, slot_sizes, assign)

    kwargs = {"trace": True} if _trace else {}
    res = run_bass_kernel_spmd(nc, in_maps, core_ids=list(range(NCORES)), **kwargs)
    _last_results = res

    out = np.empty((B, SQ, D), np.float32)
    for c in range(NCORES):
        o = res.results[c]["o"]  # [SLOTS_PER_CORE, VA_W, QCH]
        for s in range(SLOTS_PER_CORE):
            b, h, _ = assign[c][s]
            num = o[s, :D, :]          # [D, QCH]
            den = o[s, D, :]           # [QCH]
            out[b, h * QCH : (h + 1) * QCH, :] = (num / den).T
    return out
